# revision 7
# baseline (speedup 1.0000x reference)
"""Causal self-attention (B=2, T=4096, C=768, H=12, Dh=64) on 8 TRN2 NeuronCores.

Sharding: batch x head-groups. Core c handles batch b = c//4 and the 3 heads
hh = 3*(c%4) .. hh+2 of that batch (data parallel on B, tensor parallel on
heads for the qkv / out projections). Each core computes a partial output
y_c = attn_out(heads) @ W_out[head rows]; the host sums the 4 partials per
batch and adds b_out.

Device-side layout (per core, identical SPMD program):
  xt    [768, 4096]  x[b].T (host pre-transposed so C lands on partitions)
  wqkv  [768, 576]   columns permuted to [q0 q1 | k0 k1 | q2 k2 | v0 v1 v2]
  bqkv  [576]        same permutation
  wout  [192, 768]   rows for this core's heads
  y     [4096, 768]  partial output (no b_out)

Projection phase produces:
  A  = [q0|q1]^T  [128, T]   (head0 on partitions 0-63, head1 on 64-127)
  B_ = [k0|k1]^T  [128, T]
  Q2 = [q2|q2]^T  [128, T]   (head2 q mirrored onto both halves)
  K2 = [k2|k2]^T  [128, T]   (head2 k mirrored onto both halves)
  v_st [128, 32, 3, 65]      v in [token, d] layout per 128-token block,
                             col 64 = 1.0 (gives softmax row-sums for free)

Attention: the PE array supports row-tiled concurrency — two K=64 matmuls on
disjoint partition halves (tile_position auto-derived from base partitions)
stream simultaneously. Per q-super: loop A pairs h0 (rows 0-63) with h1
(rows 64-127) per k-block; loop B pairs h2 with itself across consecutive
k-blocks via the Q2/K2 mirrors. s^T tiles [128 k, SUP q] via
matmul(lhsT=k_chunk, rhs=q_super); exp splits across ACT (native, with
additive -1e5 causal mask on the diagonal 128-block) and DVE (Schraudolph
int32 bit-trick + multiplicative 0/1 triangle on the diagonal block);
av^T [65, SUP] accumulates matmul(lhsT=v_aug, rhs=p^T) over k-blocks. Row 64
of av^T is the softmax denominator; normalization evacuates av via ACT copy,
reshapes the denominator row to [8, 128] by DMA so reciprocal_approx_fast
runs wide (194ns vs 1.1us on one lane), broadcasts via gpsimd, multiplies on
DVE into attnT. Out-projection contracts attnT (K=128 + K=64 matmuls) with
wout; y evacuates PSUM via ACT (DMA cannot read PSUM).

All matmuls run in float32r (~1.5e-4 rel err, 4x faster than fp32).
"""

import contextlib
import math

import numpy as np

import concourse.bass as bass
import concourse.tile as tile
from concourse import bacc, mybir
from concourse.bass_utils import run_bass_kernel_spmd

F32 = mybir.dt.float32
F32R = mybir.dt.float32r
BF16 = mybir.dt.bfloat16
I16 = mybir.dt.int16

# Schraudolph exp constants, int16/bfloat16 domain (the BIR verifier requires
# f32r-rounded producers for f32r matmul inputs, so pt is bf16 instead):
# exp(s*SCALE) ~= bitcast_bf16(int16(As*s + Bs))
LOG2E = 1.4426950408889634
SCH_A = 128.0 * LOG2E  # * SCALE applied at use site
SCH_B = 128.0 * (127.0 - 0.04367744890362246)

T = 4096
C = 768
H = 12
DH = 64
HPC = 3  # heads per core
NCORES = 8
SUP = 1024  # q-super width
NSUP = T // SUP
KB = 128  # k-block
NKB = T // KB
SCALE = 1.0 / math.sqrt(DH)

TRACE = False
LAST_RESULT = None
_PROG = None


DEFAULT_CFG = {
    "st_bufs": 2,
    "av_bufs": 2,
    "pt_bufs": 4,
    "repeat": 1,  # hardware For_i repetitions of the whole body (benchmarking)
}


def build_program(debug=False, cfg=None):
    cfg = {**DEFAULT_CFG, **(cfg or {})}
    nc = bacc.Bacc("TRN2", target_bir_lowering=False, debug=False)
    xt_d = nc.dram_tensor("xt", [C, T], F32, kind="ExternalInput").ap()
    wqkv_d = nc.dram_tensor("wqkv", [C, 576], F32, kind="ExternalInput").ap()
    bqkv_d = nc.dram_tensor("bqkv", [576], F32, kind="ExternalInput").ap()
    wout_d = nc.dram_tensor("wout", [192, C], F32, kind="ExternalInput").ap()
    y_d = nc.dram_tensor("y", [T, C], F32, kind="ExternalOutput").ap()

    with tile.TileContext(nc) as tc:
        with tc.tile_pool(name="res", bufs=1) as res:
            A = res.tile([128, T], F32R, tag="A")
            B_ = res.tile([128, T], F32R, tag="B")
            Q2 = res.tile([128, T], F32R, tag="Q2")
            K2 = res.tile([128, T], F32R, tag="K2")
            v_st = res.tile([128, NKB, HPC, DH + 1], BF16, tag="v_st")
            wo01 = res.tile([128, C], F32R, tag="wo01")
            wo2 = res.tile([64, C], F32R, tag="wo2")
            at01 = res.tile([128, T], F32R, tag="at01")  # [h0 d | h1 d] x q
            at2 = res.tile([64, T], F32R, tag="at2")

            # multiplicative causal triangle for the diagonal 128-block
            # (1.0 where q >= k else 0.0) — applied after Schraudolph exp
            tri01 = res.tile([128, KB], BF16, tag="tri01")
            nc.gpsimd.memset(tri01[:], 1.0)
            nc.gpsimd.affine_select(
                out=tri01[:],
                in_=tri01[:],
                compare_op=mybir.AluOpType.is_ge,
                fill=0.0,
                base=0,
                pattern=[[1, KB]],
                channel_multiplier=-1,
            )
            # additive variant (0 / -1e5) applied before ACT exp
            maskadd = res.tile([128, KB], F32, tag="maskadd")
            nc.gpsimd.memset(maskadd[:], 0.0)
            nc.gpsimd.affine_select(
                out=maskadd[:],
                in_=maskadd[:],
                compare_op=mybir.AluOpType.is_ge,
                fill=-1e5,
                base=0,
                pattern=[[1, KB]],
                channel_multiplier=-1,
            )
            nc.vector.memset(v_st[:, :, :, DH : DH + 1], 1.0)

            rep_ctx = (
                tc.For_i(0, cfg["repeat"], 1)
                if cfg.get("repeat", 1) > 1
                else contextlib.nullcontext()
            )
            rep_ctx.__enter__()

            # ---------------- Phase 1: projections ----------------
            with (
                tc.tile_pool(name="p1", bufs=1) as p1,
                tc.tile_pool(name="xts", bufs=3) as xpool,
                tc.tile_pool(name="pps", bufs=2, space="PSUM") as pps,
                tc.tile_pool(name="vps", bufs=2, space="PSUM") as vps,
            ):
                # 640 = 576 + 64 zero pad so the v-projection moving dim is
                # 256 (fp32r matmuls with N < 256 run at 1/4 rate)
                wq_sb = p1.tile([128, 6, 640], F32R, tag="wq")
                bias_qk = p1.tile([128, 3], F32, tag="bqk")
                bias_v = p1.tile([128, 192], F32, tag="bv")
                bias_v_row = p1.tile([1, 192], F32, tag="bvr")

                for ci in range(6):
                    nc.sync.dma_start(
                        wq_sb[:, ci, 0:576],
                        wqkv_d[ci * 128 : (ci + 1) * 128, :].bitcast(F32R),
                    )
                nc.vector.memset(wq_sb[:, :, 576:640].bitcast(F32), 0.0)
                # wout is needed only in phase 2 — load it behind the weights
                nc.sync.dma_start(wo01[:], wout_d[0:128, :].bitcast(F32R))
                nc.sync.dma_start(wo2[:], wout_d[128:192, :].bitcast(F32R))
                for m in range(3):
                    nc.sync.dma_start(
                        bias_qk[:, m : m + 1],
                        bqkv_d[m * 128 : (m + 1) * 128].rearrange("(p b) -> p b", b=1),
                    )
                nc.sync.dma_start(
                    bias_v_row[0:1, :],
                    bqkv_d[384:576].rearrange("(b f) -> b f", b=1),
                )
                nc.gpsimd.partition_broadcast(bias_v[:], bias_v_row[0:1, :])

                for ts in range(T // 512):
                    xts = xpool.tile([128, 6, 512], F32R, tag="xts")
                    nc.sync.dma_start(
                        xts[:],
                        xt_d[:, ts * 512 : (ts + 1) * 512]
                        .rearrange("(ci p) n -> p ci n", p=128)
                        .bitcast(F32R),
                    )
                    col0 = ts * 512
                    # q/k rows (transposed layout): psum [qkv-rows, tokens]
                    for m in range(3):
                        psq = pps.tile([128, 512], F32, tag="psq")
                        for ci in range(6):
                            nc.tensor.matmul(
                                psq[:],
                                wq_sb[:, ci, m * 128 : (m + 1) * 128],
                                xts[:, ci, :],
                                start=(ci == 0),
                                stop=(ci == 5),
                            )
                        if m < 2:
                            dest = A if m == 0 else B_
                            nc.vector.tensor_scalar_add(
                                out=dest[:, col0 : col0 + 512],
                                in0=psq[:],
                                scalar1=bias_qk[:, m : m + 1],
                            )
                        else:
                            # m==2 block is [q2 (rows 0-63) | k2 (rows 64-127)]:
                            # q2 to Q2 lower half, k2 to K2 upper half; mirrors
                            # fill the other halves after the loop.
                            nc.vector.tensor_scalar_add(
                                out=Q2[0:64, col0 : col0 + 512],
                                in0=psq[0:64, :],
                                scalar1=bias_qk[0:64, 2:3],
                            )
                            nc.vector.tensor_scalar_add(
                                out=K2[64:128, col0 : col0 + 512],
                                in0=psq[64:128, :],
                                scalar1=bias_qk[64:128, 2:3],
                            )
                    # v in [token, d] layout: psum [tokens, 3*64 (+64 pad)]
                    for tb in range(4):
                        psv = vps.tile([128, 256], F32, tag="psv")
                        for ci in range(6):
                            nc.tensor.matmul(
                                psv[:],
                                xts[:, ci, tb * 128 : (tb + 1) * 128],
                                wq_sb[:, ci, 384:640],
                                start=(ci == 0),
                                stop=(ci == 5),
                            )
                        kb = ts * 4 + tb
                        nc.vector.tensor_tensor(
                            out=v_st[:, kb, :, 0:DH],
                            in0=psv[:, 0:192].rearrange("p (h d) -> p h d", h=HPC),
                            in1=bias_v[:].rearrange("p (h d) -> p h d", h=HPC),
                            op=mybir.AluOpType.add,
                        )

            # head-2 mirrors so paired s^T matmuls can stream the same SBUF
            # addresses on both partition halves
            nc.sync.dma_start(Q2[64:128, :], Q2[0:64, :])
            nc.sync.dma_start(K2[0:64, :], K2[64:128, :])

            # ---------------- Phase 2: attention + out-projection ----------------
            with (
                tc.tile_pool(name="stps", bufs=cfg["st_bufs"], space="PSUM") as stps,
                tc.tile_pool(name="avps", bufs=cfg["av_bufs"], space="PSUM") as avps,
                tc.tile_pool(name="ptp", bufs=cfg["pt_bufs"]) as ptp,
                tc.tile_pool(name="nrm", bufs=2) as nrm,
                tc.tile_pool(name="ysb", bufs=2) as ypool,
            ):

                def st_mms(st, lhs, kb, rhs, q0, ext0):
                    """s^T matmuls for one (head-half, k-block), split at PSUM
                    bank boundaries."""
                    c = ext0
                    while c < SUP:
                        ce = min((c // 512 + 1) * 512, SUP)
                        nc.tensor.matmul(
                            st[:, c:ce],
                            lhs[:, kb * KB : (kb + 1) * KB],
                            rhs[:, q0 + c : q0 + ce],
                            start=True,
                            stop=True,
                        )
                        c = ce

                def exp_chunk(st, pt, t, ext0, eng):
                    """pt[:, ext0:SUP] = exp(SCALE * st[:, ext0:SUP]); t >= 0
                    marks a diagonal k-block whose causal mask sits at cols
                    [ext0, ext0+KB)."""
                    if eng == "act":
                        if t >= 0:
                            nc.vector.tensor_tensor(
                                out=st[:, ext0 : ext0 + KB],
                                in0=st[:, ext0 : ext0 + KB],
                                in1=maskadd[:],
                                op=mybir.AluOpType.add,
                            )
                        nc.scalar.activation(
                            out=pt[:, ext0:SUP],
                            in_=st[:, ext0:SUP],
                            func=mybir.ActivationFunctionType.Exp,
                            bias=0.0,
                            scale=SCALE,
                        )
                    else:
                        # Schraudolph on DVE: int32(A*s + B) bit-pattern is
                        # exp(s*SCALE) to ~2%; diagonal block then multiplied
                        # by the 0/1 triangle (masked entries are finite
                        # positives, so 0 * p = 0).
                        nc.vector.tensor_scalar(
                            out=pt[:, ext0:SUP].bitcast(I16),
                            in0=st[:, ext0:SUP],
                            scalar1=float(SCH_A * SCALE),
                            scalar2=float(SCH_B),
                            op0=mybir.AluOpType.mult,
                            op1=mybir.AluOpType.add,
                        )
                        if t >= 0:
                            nc.vector.tensor_tensor(
                                out=pt[:, ext0 : ext0 + KB],
                                in0=pt[:, ext0 : ext0 + KB],
                                in1=tri01[:],
                                op=mybir.AluOpType.mult,
                            )

                def av_acc(av, pt, h, kb, qs, ext0):
                    nkb = (qs + 1) * (SUP // KB)
                    last_r0 = qs * 8 + 512 // KB - 1  # last kb touching cols [0,512)
                    c = ext0
                    while c < SUP:
                        ce = min((c // 512 + 1) * 512, SUP)
                        stop_kb = last_r0 if ce <= 512 else nkb - 1
                        nc.tensor.matmul(
                            av[:, c:ce],
                            v_st[:, kb, h, :],
                            pt[:, c:ce],
                            start=(kb == 0),
                            stop=(kb == stop_kb),
                        )
                        c = ce

                def norm_head(qs, h, av):
                    """rows 0-63 of av divided by row 64, into attnT storage.
                    ACT evacuates av (DMA cannot read PSUM); the denominator
                    row is DMA-reshaped to [8, 128] so the reciprocal runs on
                    8 lanes instead of 1."""
                    q0 = qs * SUP
                    stg = nrm.tile([65, SUP], F32, tag="stg")
                    nc.scalar.copy(stg[:], av[:])
                    den8 = nrm.tile([8, KB], F32, tag="den8")
                    nc.sync.dma_start(
                        den8[:],
                        stg[64:65, :].rearrange("r (a b) -> r a b", a=8),
                    )
                    rec8 = nrm.tile([8, KB], F32, tag="rec8")
                    nc.vector.reciprocal_approx_fast(out=rec8[:], in_=den8[:])
                    rec = nrm.tile([1, SUP], F32, tag="rec")
                    nc.sync.dma_start(
                        rec[0:1, :].rearrange("r (a b) -> r a b", a=8),
                        rec8[:],
                    )
                    recb = nrm.tile([64, SUP], F32, tag="recb")
                    nc.gpsimd.partition_broadcast(recb[:], rec[0:1, :])
                    if h == 0:
                        dest = at01[0:64, q0 : q0 + SUP]
                    elif h == 2:
                        dest = at2[:, q0 : q0 + SUP]
                    else:
                        # h1 rows belong at partitions 64-127 of at01; DVE
                        # can't shift partitions, so stage + DMA.
                        h1s = nrm.tile([64, SUP], F32R, tag="h1s")
                        dest = h1s[:]
                    nc.vector.tensor_tensor(
                        out=dest,
                        in0=stg[0:64, :],
                        in1=recb[:],
                        op=mybir.AluOpType.mult,
                    )
                    if h == 1:
                        nc.sync.dma_start(at01[64:128, q0 : q0 + SUP], h1s[:])

                def out_proj(qs):
                    q0 = qs * SUP
                    for tb in range(SUP // 128):
                        tcol = q0 + tb * 128
                        yps = stps.tile([128, SUP], F32, tag="st")
                        for rs, re in ((0, 512), (512, C)):
                            nc.tensor.matmul(
                                yps[:, rs:re],
                                at01[:, tcol : tcol + 128],
                                wo01[:, rs:re],
                                start=True,
                                stop=False,
                            )
                            nc.tensor.matmul(
                                yps[:, rs:re],
                                at2[:, tcol : tcol + 128],
                                wo2[:, rs:re],
                                start=False,
                                stop=True,
                            )
                        y_sb = ypool.tile([128, C], F32, tag="ysb")
                        nc.scalar.copy(y_sb[:], yps[:, 0:C])
                        nc.sync.dma_start(y_d[tcol : tcol + 128, :], y_sb[:])

                def pair_round(kb_lo, kb_hi, qs, t, lhsL, rhsL, lhsH, rhsH, avL, avH,
                               hL, hH, engL, engH):
                    """One round of two row-tiled concurrent s^T matmuls on
                    opposite partition halves, exp on both engines, then the
                    (serial, K=128) av accumulations."""
                    q0 = qs * SUP
                    ext0 = max(t, 0) * KB
                    stA = stps.tile([128, SUP], F32, tag="st")
                    stB = stps.tile([128, SUP], F32, tag="st")
                    # interleave at 512-col granularity so the two halves'
                    # matmuls sit adjacent in the PE queue
                    c = ext0
                    while c < SUP:
                        ce = min((c // 512 + 1) * 512, SUP)
                        nc.tensor.matmul(
                            stA[:, c:ce],
                            lhsL[:, kb_lo * KB : (kb_lo + 1) * KB],
                            rhsL[:, q0 + c : q0 + ce],
                            start=True,
                            stop=True,
                        )
                        nc.tensor.matmul(
                            stB[:, c:ce],
                            lhsH[:, kb_hi * KB : (kb_hi + 1) * KB],
                            rhsH[:, q0 + c : q0 + ce],
                            start=True,
                            stop=True,
                        )
                        c = ce
                    ptA = ptp.tile([128, SUP], BF16, tag="pt")
                    ptB = ptp.tile([128, SUP], BF16, tag="pt")
                    exp_chunk(stA, ptA, t, ext0, engL)
                    exp_chunk(stB, ptB, t if kb_hi == kb_lo else -1, ext0, engH)
                    av_acc(avL, ptA, hL, kb_lo, qs, ext0)
                    av_acc(avH, ptB, hH, kb_hi, qs, ext0)

                for qs in range(NSUP):
                    q0 = qs * SUP
                    nkb = (qs + 1) * (SUP // KB)
                    koff = qs * 8  # first diagonal k-block
                    # ---- loop A: h0 (rows 0-63) paired with h1 (rows 64-127)
                    av0 = avps.tile([65, SUP], F32, tag="av")
                    av1 = avps.tile([65, SUP], F32, tag="av")
                    for kb in range(nkb):
                        t = kb - koff
                        pair_round(
                            kb, kb, qs, t,
                            B_[0:64, :], A[0:64, :], B_[64:128, :], A[64:128, :],
                            av0, av1, 0, 1,
                            "act", "dve" if qs >= 1 else "act",
                        )
                    norm_head(qs, 0, av0)
                    norm_head(qs, 1, av1)
                    # ---- loop B: h2 self-paired across consecutive k-blocks
                    # (av2 reuses av0's PSUM slot, so the h0/h1 norms must be
                    # emitted before it)
                    av2 = avps.tile([65, SUP], F32, tag="av")
                    for kb in range(0, koff, 2):
                        pair_round(
                            kb, kb + 1, qs, -1,
                            K2[0:64, :], Q2[0:64, :], K2[64:128, :], Q2[64:128, :],
                            av2, av2, 2, 2, "act", "dve",
                        )
                    # diagonal k-blocks: serial, alternating halves (weights
                    # load into the idle half while the other streams)
                    for i, kb in enumerate(range(koff, nkb)):
                        t = kb - koff
                        ext0 = t * KB
                        st2 = stps.tile([128, SUP], F32, tag="st")
                        lhs = K2[0:64, :] if i % 2 == 0 else K2[64:128, :]
                        rhs = Q2[0:64, :] if i % 2 == 0 else Q2[64:128, :]
                        st_mms(st2, lhs, kb, rhs, q0, ext0)
                        pt2 = ptp.tile([128, SUP], BF16, tag="pt")
                        eng = "dve" if (qs >= 1 and i % 2 == 0) else "act"
                        exp_chunk(st2, pt2, t, ext0, eng)
                        av_acc(av2, pt2, 2, kb, qs, ext0)
                    norm_head(qs, 2, av2)
                    if qs >= 1:
                        out_proj(qs - 1)
                out_proj(NSUP - 1)
            rep_ctx.__exit__(None, None, None)

    nc.compile()
    return nc


def shard_inputs(x, W_qkv, b_qkv, W_out, b_out):
    """Build the per-core input maps (host-side sharding)."""
    x = np.asarray(x, dtype=np.float32)
    W_qkv = np.asarray(W_qkv, dtype=np.float32)
    b_qkv = np.asarray(b_qkv, dtype=np.float32)
    W_out = np.asarray(W_out, dtype=np.float32)
    in_maps = []
    for c in range(NCORES):
        b = c // 4
        hh = (c % 4) * HPC
        h0, h1, h2 = hh, hh + 1, hh + 2

        def qcols(h):
            return list(range(h * DH, (h + 1) * DH))

        def kcols(h):
            return list(range(C + h * DH, C + (h + 1) * DH))

        def vcols(h):
            return list(range(2 * C + h * DH, 2 * C + (h + 1) * DH))

        perm = (
            qcols(h0) + qcols(h1) + kcols(h0) + kcols(h1) + qcols(h2) + kcols(h2)
            + vcols(h0) + vcols(h1) + vcols(h2)
        )
        in_maps.append(
            {
                "xt": np.ascontiguousarray(x[b].T),
                "wqkv": np.ascontiguousarray(W_qkv[:, perm]),
                "bqkv": np.ascontiguousarray(b_qkv[perm]),
                "wout": np.ascontiguousarray(W_out[hh * DH : (hh + HPC) * DH, :]),
            }
        )
    return in_maps


def kernel(x, W_qkv, b_qkv, W_out, b_out):
    global _PROG, LAST_RESULT
    if _PROG is None:
        _PROG = build_program()
    nc = _PROG
    in_maps = shard_inputs(x, W_qkv, b_qkv, W_out, b_out)
    res = run_bass_kernel_spmd(nc, in_maps, list(range(NCORES)), trace=TRACE)
    LAST_RESULT = res
    b_out = np.asarray(b_out, dtype=np.float32)
    y = np.zeros((2, T, C), dtype=np.float32)
    for c in range(NCORES):
        y[c // 4] += res.results[c]["y"]
    y += b_out[None, None, :]
    return y


# revision 8
# speedup vs baseline: 1.0537x; 1.0537x over previous
"""Causal self-attention (B=2, T=4096, C=768, H=12, Dh=64) on 8 TRN2 NeuronCores.

Sharding: batch x head-groups. Core c handles batch b = c//4 and the 3 heads
hh = 3*(c%4) .. hh+2 of that batch (data parallel on B, tensor parallel on
heads for the qkv / out projections). Each core computes a partial output
y_c = attn_out(heads) @ W_out[head rows]; the host sums the 4 partials per
batch and adds b_out.

Device-side layout (per core, identical SPMD program):
  xt    [768, 4096]  x[b].T (host pre-transposed so C lands on partitions)
  wqkv  [768, 576]   columns permuted to [q0 q1 | k0 k1 | q2 k2 | v0 v1 v2]
  bqkv  [576]        same permutation
  wout  [192, 768]   rows for this core's heads
  y     [4096, 768]  partial output (no b_out)

Projection phase produces:
  A  = [q0|q1]^T  [128, T]   (head0 on partitions 0-63, head1 on 64-127)
  B_ = [k0|k1]^T  [128, T]
  Q2 = [q2|q2]^T  [128, T]   (head2 q mirrored onto both halves)
  K2 = [k2|k2]^T  [128, T]   (head2 k mirrored onto both halves)
  v_st [128, 32, 3, 65]      v in [token, d] layout per 128-token block,
                             col 64 = 1.0 (gives softmax row-sums for free)

Attention: the PE array supports row-tiled concurrency — two K=64 matmuls on
disjoint partition halves (tile_position auto-derived from base partitions)
stream simultaneously. Per q-super: loop A pairs h0 (rows 0-63) with h1
(rows 64-127) per k-block; loop B pairs h2 with itself across consecutive
k-blocks via the Q2/K2 mirrors. s^T tiles [128 k, SUP q] via
matmul(lhsT=k_chunk, rhs=q_super); exp splits across ACT (native, with
additive -1e5 causal mask on the diagonal 128-block) and DVE (Schraudolph
int32 bit-trick + multiplicative 0/1 triangle on the diagonal block);
av^T [65, SUP] accumulates matmul(lhsT=v_aug, rhs=p^T) over k-blocks. Row 64
of av^T is the softmax denominator; normalization evacuates av via ACT copy,
reshapes the denominator row to [8, 128] by DMA so reciprocal_approx_fast
runs wide (194ns vs 1.1us on one lane), broadcasts via gpsimd, multiplies on
DVE into attnT. Out-projection contracts attnT (K=128 + K=64 matmuls) with
wout; y evacuates PSUM via ACT (DMA cannot read PSUM).

All matmuls run in bfloat16 (1 cycle/row on the PE; f32r measures 2).
"""

import contextlib
import math

import ml_dtypes
import numpy as np

BF16_NP = ml_dtypes.bfloat16

import concourse.bass as bass
import concourse.tile as tile
from concourse import bacc, mybir
from concourse.bass_utils import run_bass_kernel_spmd

F32 = mybir.dt.float32
F32R = mybir.dt.float32r
BF16 = mybir.dt.bfloat16
I16 = mybir.dt.int16

# Schraudolph exp constants, int16/bfloat16 domain (the BIR verifier requires
# f32r-rounded producers for f32r matmul inputs, so pt is bf16 instead):
# exp(s*SCALE) ~= bitcast_bf16(int16(As*s + Bs))
LOG2E = 1.4426950408889634
SCH_A = 128.0 * LOG2E  # * SCALE applied at use site
SCH_B = 128.0 * (127.0 - 0.04367744890362246)

T = 4096
C = 768
H = 12
DH = 64
HPC = 3  # heads per core
NCORES = 8
SUP = 1024  # q-super width
NSUP = T // SUP
KB = 128  # k-block
NKB = T // KB
SCALE = 1.0 / math.sqrt(DH)

TRACE = False
LAST_RESULT = None
_PROG = None


DEFAULT_CFG = {
    "st_bufs": 2,
    "av_bufs": 2,
    "pt_bufs": 4,
    "repeat": 1,  # hardware For_i repetitions of the whole body (benchmarking)
}


def build_program(debug=False, cfg=None):
    cfg = {**DEFAULT_CFG, **(cfg or {})}
    nc = bacc.Bacc("TRN2", target_bir_lowering=False, debug=False)
    xt_d = nc.dram_tensor("xt", [C, T], BF16, kind="ExternalInput").ap()
    wqkv_d = nc.dram_tensor("wqkv", [C, 576], BF16, kind="ExternalInput").ap()
    bqkv_d = nc.dram_tensor("bqkv", [576], F32, kind="ExternalInput").ap()
    wout_d = nc.dram_tensor("wout", [192, C], BF16, kind="ExternalInput").ap()
    y_d = nc.dram_tensor("y", [T, C], F32, kind="ExternalOutput").ap()

    with tile.TileContext(nc) as tc:
        with tc.tile_pool(name="res", bufs=1) as res:
            A = res.tile([128, T], BF16, tag="A")
            B_ = res.tile([128, T], BF16, tag="B")
            Q2 = res.tile([128, T], BF16, tag="Q2")
            K2 = res.tile([128, T], BF16, tag="K2")
            v_st = res.tile([128, NKB, HPC, DH + 1], BF16, tag="v_st")
            wo01 = res.tile([128, C], BF16, tag="wo01")
            wo2 = res.tile([64, C], BF16, tag="wo2")
            at01 = res.tile([128, T], BF16, tag="at01")  # [h0 d | h1 d] x q
            at2 = res.tile([64, T], BF16, tag="at2")

            # multiplicative causal triangle for the diagonal 128-block
            # (1.0 where q >= k else 0.0) — applied after Schraudolph exp
            tri01 = res.tile([128, KB], BF16, tag="tri01")
            nc.gpsimd.memset(tri01[:], 1.0)
            nc.gpsimd.affine_select(
                out=tri01[:],
                in_=tri01[:],
                compare_op=mybir.AluOpType.is_ge,
                fill=0.0,
                base=0,
                pattern=[[1, KB]],
                channel_multiplier=-1,
            )
            # additive variant (0 / -1e5) applied before ACT exp
            maskadd = res.tile([128, KB], F32, tag="maskadd")
            nc.gpsimd.memset(maskadd[:], 0.0)
            nc.gpsimd.affine_select(
                out=maskadd[:],
                in_=maskadd[:],
                compare_op=mybir.AluOpType.is_ge,
                fill=-1e5,
                base=0,
                pattern=[[1, KB]],
                channel_multiplier=-1,
            )
            nc.vector.memset(v_st[:, :, :, DH : DH + 1], 1.0)

            rep_ctx = (
                tc.For_i(0, cfg["repeat"], 1)
                if cfg.get("repeat", 1) > 1
                else contextlib.nullcontext()
            )
            rep_ctx.__enter__()

            # ---------------- Phase 1: projections ----------------
            with (
                tc.tile_pool(name="p1", bufs=1) as p1,
                tc.tile_pool(name="xts", bufs=3) as xpool,
                tc.tile_pool(name="pps", bufs=2, space="PSUM") as pps,
                tc.tile_pool(name="vps", bufs=2, space="PSUM") as vps,
            ):
                # 640 = 576 + 64 zero pad so the v-projection moving dim is
                # 256 (fp32r matmuls with N < 256 run at 1/4 rate)
                wq_sb = p1.tile([128, 6, 640], BF16, tag="wq")
                bias_qk = p1.tile([128, 3], F32, tag="bqk")
                bias_v = p1.tile([128, 192], F32, tag="bv")
                bias_v_row = p1.tile([1, 192], F32, tag="bvr")

                for ci in range(6):
                    nc.sync.dma_start(
                        wq_sb[:, ci, 0:576],
                        wqkv_d[ci * 128 : (ci + 1) * 128, :],
                    )
                nc.vector.memset(wq_sb[:, :, 576:640], 0.0)
                # wout is needed only in phase 2 — load it behind the weights
                nc.sync.dma_start(wo01[:], wout_d[0:128, :])
                nc.sync.dma_start(wo2[:], wout_d[128:192, :])
                for m in range(3):
                    nc.sync.dma_start(
                        bias_qk[:, m : m + 1],
                        bqkv_d[m * 128 : (m + 1) * 128].rearrange("(p b) -> p b", b=1),
                    )
                nc.sync.dma_start(
                    bias_v_row[0:1, :],
                    bqkv_d[384:576].rearrange("(b f) -> b f", b=1),
                )
                nc.gpsimd.partition_broadcast(bias_v[:], bias_v_row[0:1, :])

                for ts in range(T // 512):
                    xts = xpool.tile([128, 6, 512], BF16, tag="xts")
                    nc.sync.dma_start(
                        xts[:],
                        xt_d[:, ts * 512 : (ts + 1) * 512]
                        .rearrange("(ci p) n -> p ci n", p=128),
                    )
                    col0 = ts * 512
                    # q/k rows (transposed layout): psum [qkv-rows, tokens]
                    for m in range(3):
                        psq = pps.tile([128, 512], F32, tag="psq")
                        for ci in range(6):
                            nc.tensor.matmul(
                                psq[:],
                                wq_sb[:, ci, m * 128 : (m + 1) * 128],
                                xts[:, ci, :],
                                start=(ci == 0),
                                stop=(ci == 5),
                            )
                        if m < 2:
                            dest = A if m == 0 else B_
                            nc.vector.tensor_scalar_add(
                                out=dest[:, col0 : col0 + 512],
                                in0=psq[:],
                                scalar1=bias_qk[:, m : m + 1],
                            )
                        else:
                            # m==2 block is [q2 (rows 0-63) | k2 (rows 64-127)]:
                            # q2 to Q2 lower half, k2 to K2 upper half; mirrors
                            # fill the other halves after the loop.
                            nc.vector.tensor_scalar_add(
                                out=Q2[0:64, col0 : col0 + 512],
                                in0=psq[0:64, :],
                                scalar1=bias_qk[0:64, 2:3],
                            )
                            nc.vector.tensor_scalar_add(
                                out=K2[64:128, col0 : col0 + 512],
                                in0=psq[64:128, :],
                                scalar1=bias_qk[64:128, 2:3],
                            )
                    # v in [token, d] layout: psum [tokens, 3*64 (+64 pad)]
                    for tb in range(4):
                        psv = vps.tile([128, 256], F32, tag="psv")
                        for ci in range(6):
                            nc.tensor.matmul(
                                psv[:],
                                xts[:, ci, tb * 128 : (tb + 1) * 128],
                                wq_sb[:, ci, 384:640],
                                start=(ci == 0),
                                stop=(ci == 5),
                            )
                        kb = ts * 4 + tb
                        nc.vector.tensor_tensor(
                            out=v_st[:, kb, :, 0:DH],
                            in0=psv[:, 0:192].rearrange("p (h d) -> p h d", h=HPC),
                            in1=bias_v[:].rearrange("p (h d) -> p h d", h=HPC),
                            op=mybir.AluOpType.add,
                        )

            # head-2 mirrors so paired s^T matmuls can stream the same SBUF
            # addresses on both partition halves
            nc.sync.dma_start(Q2[64:128, :], Q2[0:64, :])
            nc.sync.dma_start(K2[0:64, :], K2[64:128, :])

            # ---------------- Phase 2: attention + out-projection ----------------
            with (
                tc.tile_pool(name="stps", bufs=cfg["st_bufs"], space="PSUM") as stps,
                tc.tile_pool(name="avps", bufs=cfg["av_bufs"], space="PSUM") as avps,
                tc.tile_pool(name="ptp", bufs=cfg["pt_bufs"]) as ptp,
                tc.tile_pool(name="nrm", bufs=2) as nrm,
                tc.tile_pool(name="ysb", bufs=2) as ypool,
            ):

                def st_mms(st, lhs, kb, rhs, q0, ext0):
                    """s^T matmuls for one (head-half, k-block), split at PSUM
                    bank boundaries."""
                    c = ext0
                    while c < SUP:
                        ce = min((c // 512 + 1) * 512, SUP)
                        nc.tensor.matmul(
                            st[:, c:ce],
                            lhs[:, kb * KB : (kb + 1) * KB],
                            rhs[:, q0 + c : q0 + ce],
                            start=True,
                            stop=True,
                        )
                        c = ce

                def exp_chunk(st, pt, t, ext0, eng):
                    """pt[:, ext0:SUP] = exp(SCALE * st[:, ext0:SUP]); t >= 0
                    marks a diagonal k-block whose causal mask sits at cols
                    [ext0, ext0+KB)."""
                    if eng == "act":
                        if t >= 0:
                            nc.vector.tensor_tensor(
                                out=st[:, ext0 : ext0 + KB],
                                in0=st[:, ext0 : ext0 + KB],
                                in1=maskadd[:],
                                op=mybir.AluOpType.add,
                            )
                        nc.scalar.activation(
                            out=pt[:, ext0:SUP],
                            in_=st[:, ext0:SUP],
                            func=mybir.ActivationFunctionType.Exp,
                            bias=0.0,
                            scale=SCALE,
                        )
                    else:
                        # Schraudolph on DVE: int32(A*s + B) bit-pattern is
                        # exp(s*SCALE) to ~2%; diagonal block then multiplied
                        # by the 0/1 triangle (masked entries are finite
                        # positives, so 0 * p = 0).
                        nc.vector.tensor_scalar(
                            out=pt[:, ext0:SUP].bitcast(I16),
                            in0=st[:, ext0:SUP],
                            scalar1=float(SCH_A * SCALE),
                            scalar2=float(SCH_B),
                            op0=mybir.AluOpType.mult,
                            op1=mybir.AluOpType.add,
                        )
                        if t >= 0:
                            nc.vector.tensor_tensor(
                                out=pt[:, ext0 : ext0 + KB],
                                in0=pt[:, ext0 : ext0 + KB],
                                in1=tri01[:],
                                op=mybir.AluOpType.mult,
                            )

                def av_acc(av, pt, h, kb, qs, ext0):
                    nkb = (qs + 1) * (SUP // KB)
                    last_r0 = qs * 8 + 512 // KB - 1  # last kb touching cols [0,512)
                    c = ext0
                    while c < SUP:
                        ce = min((c // 512 + 1) * 512, SUP)
                        stop_kb = last_r0 if ce <= 512 else nkb - 1
                        nc.tensor.matmul(
                            av[:, c:ce],
                            v_st[:, kb, h, :],
                            pt[:, c:ce],
                            start=(kb == 0),
                            stop=(kb == stop_kb),
                        )
                        c = ce

                def norm_head(qs, h, av):
                    """rows 0-63 of av divided by row 64, into attnT storage.
                    ACT evacuates av (DMA cannot read PSUM); the denominator
                    row is DMA-reshaped to [8, 128] so the reciprocal runs on
                    8 lanes instead of 1."""
                    q0 = qs * SUP
                    stg = nrm.tile([65, SUP], F32, tag="stg")
                    nc.scalar.copy(stg[:], av[:])
                    den8 = nrm.tile([8, KB], F32, tag="den8")
                    nc.sync.dma_start(
                        den8[:],
                        stg[64:65, :].rearrange("r (a b) -> r a b", a=8),
                    )
                    rec8 = nrm.tile([8, KB], F32, tag="rec8")
                    nc.vector.reciprocal_approx_fast(out=rec8[:], in_=den8[:])
                    rec = nrm.tile([1, SUP], F32, tag="rec")
                    nc.sync.dma_start(
                        rec[0:1, :].rearrange("r (a b) -> r a b", a=8),
                        rec8[:],
                    )
                    recb = nrm.tile([64, SUP], F32, tag="recb")
                    nc.gpsimd.partition_broadcast(recb[:], rec[0:1, :])
                    if h == 0:
                        dest = at01[0:64, q0 : q0 + SUP]
                    elif h == 2:
                        dest = at2[:, q0 : q0 + SUP]
                    else:
                        # h1 rows belong at partitions 64-127 of at01; DVE
                        # can't shift partitions, so stage + DMA.
                        h1s = nrm.tile([64, SUP], BF16, tag="h1s")
                        dest = h1s[:]
                    nc.vector.tensor_tensor(
                        out=dest,
                        in0=stg[0:64, :],
                        in1=recb[:],
                        op=mybir.AluOpType.mult,
                    )
                    if h == 1:
                        nc.sync.dma_start(at01[64:128, q0 : q0 + SUP], h1s[:])

                def out_proj(qs):
                    q0 = qs * SUP
                    for tb in range(SUP // 128):
                        tcol = q0 + tb * 128
                        yps = stps.tile([128, SUP], F32, tag="st")
                        for rs, re in ((0, 512), (512, C)):
                            nc.tensor.matmul(
                                yps[:, rs:re],
                                at01[:, tcol : tcol + 128],
                                wo01[:, rs:re],
                                start=True,
                                stop=False,
                            )
                            nc.tensor.matmul(
                                yps[:, rs:re],
                                at2[:, tcol : tcol + 128],
                                wo2[:, rs:re],
                                start=False,
                                stop=True,
                            )
                        y_sb = ypool.tile([128, C], F32, tag="ysb")
                        nc.scalar.copy(y_sb[:], yps[:, 0:C])
                        nc.sync.dma_start(y_d[tcol : tcol + 128, :], y_sb[:])

                def pair_round(kb_lo, kb_hi, qs, t, lhsL, rhsL, lhsH, rhsH, avL, avH,
                               hL, hH, engL, engH):
                    """One round of two row-tiled concurrent s^T matmuls on
                    opposite partition halves, exp on both engines, then the
                    (serial, K=128) av accumulations."""
                    q0 = qs * SUP
                    ext0 = max(t, 0) * KB
                    stA = stps.tile([128, SUP], F32, tag="st")
                    stB = stps.tile([128, SUP], F32, tag="st")
                    # interleave at 512-col granularity so the two halves'
                    # matmuls sit adjacent in the PE queue
                    c = ext0
                    while c < SUP:
                        ce = min((c // 512 + 1) * 512, SUP)
                        nc.tensor.matmul(
                            stA[:, c:ce],
                            lhsL[:, kb_lo * KB : (kb_lo + 1) * KB],
                            rhsL[:, q0 + c : q0 + ce],
                            start=True,
                            stop=True,
                        )
                        nc.tensor.matmul(
                            stB[:, c:ce],
                            lhsH[:, kb_hi * KB : (kb_hi + 1) * KB],
                            rhsH[:, q0 + c : q0 + ce],
                            start=True,
                            stop=True,
                        )
                        c = ce
                    ptA = ptp.tile([128, SUP], BF16, tag="pt")
                    ptB = ptp.tile([128, SUP], BF16, tag="pt")
                    exp_chunk(stA, ptA, t, ext0, engL)
                    exp_chunk(stB, ptB, t if kb_hi == kb_lo else -1, ext0, engH)
                    av_acc(avL, ptA, hL, kb_lo, qs, ext0)
                    av_acc(avH, ptB, hH, kb_hi, qs, ext0)

                for qs in range(NSUP):
                    q0 = qs * SUP
                    nkb = (qs + 1) * (SUP // KB)
                    koff = qs * 8  # first diagonal k-block
                    # ---- loop A: h0 (rows 0-63) paired with h1 (rows 64-127)
                    av0 = avps.tile([65, SUP], F32, tag="av")
                    av1 = avps.tile([65, SUP], F32, tag="av")
                    for kb in range(nkb):
                        t = kb - koff
                        pair_round(
                            kb, kb, qs, t,
                            B_[0:64, :], A[0:64, :], B_[64:128, :], A[64:128, :],
                            av0, av1, 0, 1,
                            "act", "dve" if qs >= 1 else "act",
                        )
                    norm_head(qs, 0, av0)
                    norm_head(qs, 1, av1)
                    # ---- loop B: h2 self-paired across consecutive k-blocks
                    # (av2 reuses av0's PSUM slot, so the h0/h1 norms must be
                    # emitted before it)
                    av2 = avps.tile([65, SUP], F32, tag="av")
                    for kb in range(0, koff, 2):
                        pair_round(
                            kb, kb + 1, qs, -1,
                            K2[0:64, :], Q2[0:64, :], K2[64:128, :], Q2[64:128, :],
                            av2, av2, 2, 2, "act", "dve",
                        )
                    # diagonal k-blocks: serial, alternating halves (weights
                    # load into the idle half while the other streams)
                    for i, kb in enumerate(range(koff, nkb)):
                        t = kb - koff
                        ext0 = t * KB
                        st2 = stps.tile([128, SUP], F32, tag="st")
                        lhs = K2[0:64, :] if i % 2 == 0 else K2[64:128, :]
                        rhs = Q2[0:64, :] if i % 2 == 0 else Q2[64:128, :]
                        st_mms(st2, lhs, kb, rhs, q0, ext0)
                        pt2 = ptp.tile([128, SUP], BF16, tag="pt")
                        eng = "dve" if (qs >= 1 and i % 2 == 0) else "act"
                        exp_chunk(st2, pt2, t, ext0, eng)
                        av_acc(av2, pt2, 2, kb, qs, ext0)
                    norm_head(qs, 2, av2)
                    if qs >= 1:
                        out_proj(qs - 1)
                out_proj(NSUP - 1)
            rep_ctx.__exit__(None, None, None)

    nc.compile()
    return nc


def shard_inputs(x, W_qkv, b_qkv, W_out, b_out):
    """Build the per-core input maps (host-side sharding)."""
    x = np.asarray(x, dtype=np.float32)
    W_qkv = np.asarray(W_qkv, dtype=np.float32)
    b_qkv = np.asarray(b_qkv, dtype=np.float32)
    W_out = np.asarray(W_out, dtype=np.float32)
    in_maps = []
    for c in range(NCORES):
        b = c // 4
        hh = (c % 4) * HPC
        h0, h1, h2 = hh, hh + 1, hh + 2

        def qcols(h):
            return list(range(h * DH, (h + 1) * DH))

        def kcols(h):
            return list(range(C + h * DH, C + (h + 1) * DH))

        def vcols(h):
            return list(range(2 * C + h * DH, 2 * C + (h + 1) * DH))

        perm = (
            qcols(h0) + qcols(h1) + kcols(h0) + kcols(h1) + qcols(h2) + kcols(h2)
            + vcols(h0) + vcols(h1) + vcols(h2)
        )
        in_maps.append(
            {
                "xt": np.ascontiguousarray(x[b].T).astype(BF16_NP),
                "wqkv": np.ascontiguousarray(W_qkv[:, perm]).astype(BF16_NP),
                "bqkv": np.ascontiguousarray(b_qkv[perm]),
                "wout": np.ascontiguousarray(W_out[hh * DH : (hh + HPC) * DH, :]).astype(BF16_NP),
            }
        )
    return in_maps


def kernel(x, W_qkv, b_qkv, W_out, b_out):
    global _PROG, LAST_RESULT
    if _PROG is None:
        _PROG = build_program()
    nc = _PROG
    in_maps = shard_inputs(x, W_qkv, b_qkv, W_out, b_out)
    res = run_bass_kernel_spmd(nc, in_maps, list(range(NCORES)), trace=TRACE)
    LAST_RESULT = res
    b_out = np.asarray(b_out, dtype=np.float32)
    y = np.zeros((2, T, C), dtype=np.float32)
    for c in range(NCORES):
        y[c // 4] += res.results[c]["y"]
    y += b_out[None, None, :]
    return y


# revision 14
# speedup vs baseline: 1.3014x; 1.2350x over previous
"""Causal self-attention (B=2, T=4096, C=768, H=12, Dh=64) on 8 TRN2 NeuronCores.

Sharding: batch x head-groups. Core c handles batch b = c//4 and the 3 heads
hh = 3*(c%4) .. hh+2 of that batch (data parallel on B, tensor parallel on
heads for the qkv / out projections). Each core computes a partial output
y_c = attn_out(heads) @ W_out[head rows]; the host sums the 4 partials per
batch and adds b_out.

Device-side layout (per core, identical SPMD program):
  xt    [768, 4096]  x[b].T, bf16 (host pre-transposed / pre-cast)
  wqkv  [768, 576]   bf16, columns permuted to [q0 k0 | q1 k1 | q2 k2 | v0 v1 v2]
  bqkv  [576]        f32, same permutation
  wout  [192, 768]   bf16, rows for this core's heads
  y     [4096, 768]  f32 partial output (no b_out)

Projection phase produces, per head h:
  QM[:, h, :]  [128, T]  q_h^T mirrored onto both partition halves
  KM[:, h, :]  [128, T]  k_h^T mirrored onto both partition halves
  v_st [128, 32, 3, 65]  v in [token, d] layout per 128-token block,
                         col 64 = 1.0 (softmax row-sums ride along in av)

Attention: the PE array runs two K=64 matmuls concurrently when they sit on
disjoint partition halves (row tiling; tile_position is auto-derived from
base partitions — measured 2.7x on this hardware). The per-head mirrors make
consecutive k-blocks pairable: k-block kb streams on partitions 0-63 while
kb+1 streams on 64-127, reading identical SBUF addresses. Heads process
sequentially, so only one av accumulator is live: PSUM = 3 s^T slots
(2 banks each) + 1 av slot (2 banks). In the diagonal super, pair members
share the lower member's column range and a 256-wide kill+triangle mask
zeroes the upper member's out-of-range columns after exp.

exp splits per round across ACT (native exp) and DVE (Schraudolph int16
bit-trick producing bf16: exp(s/8) ~= bitcast(int16(128*log2e/8*s +
128*126.956))); both write bf16 pt, and a 0/1 triangle multiply applies the
causal mask post-exp (masked entries are finite positives). av^T [65, SUP]
accumulates matmul(lhsT=v_aug, rhs=p^T) over k-blocks; row 64 is the softmax
denominator. Normalization: ACT evacuates av to SBUF, the denominator row is
DMA-reshaped to [8, 128] so reciprocal_approx_fast runs on 8 lanes, gpsimd
broadcasts, DVE multiplies into attnT (h1 stages + DMAs to partitions
64-127). Out-projection contracts attnT (K=128 + K=64 matmuls) with wout,
reusing the s^T PSUM slots; y evacuates via ACT (DMA cannot read PSUM).

All matmul operands are bfloat16 (1 cycle/row on the PE; f32r measures 2).
"""

import contextlib
import math

import ml_dtypes
import numpy as np

import concourse.bass as bass
import concourse.tile as tile
from concourse import bacc, mybir
from concourse.bass_utils import run_bass_kernel_spmd

BF16_NP = ml_dtypes.bfloat16

F32 = mybir.dt.float32
BF16 = mybir.dt.bfloat16
I16 = mybir.dt.int16

# Schraudolph exp constants, int16/bfloat16 domain
LOG2E = 1.4426950408889634
SCH_A = 128.0 * LOG2E  # * SCALE applied at use site
SCH_B = 128.0 * (127.0 - 0.04367744890362246)

T = 4096
C = 768
H = 12
DH = 64
HPC = 3  # heads per core
NCORES = 8
SUP = 1024  # q-super width
NSUP = T // SUP
KB = 128  # k-block
NKB = T // KB
SCALE = 1.0 / math.sqrt(DH)

TRACE = False
LAST_RESULT = None
_PROG = None


DEFAULT_CFG = {
    "st_bufs": 3,
    "pt_bufs": 4,
    "eng_lo": "dve",  # exp engine for the pair's lower member (qs>=1)
    "eng_hi": "act",
    "qs0_act": True,  # force ACT exp for the all-diagonal first super
    "repeat": 1,  # hardware For_i repetitions of the whole body (benchmarking)
    # experiment knobs (sim bisection): limit supers / skip stages
    "nsup": NSUP,
    "skip_outproj": False,
    "skip_norm": False,
    "skip_exp": False,
    "phase1_only": False,
}


def build_program(debug=False, cfg=None):
    cfg = {**DEFAULT_CFG, **(cfg or {})}
    nc = bacc.Bacc("TRN2", target_bir_lowering=False, debug=False)
    xt_d = nc.dram_tensor("xt", [C, T], BF16, kind="ExternalInput").ap()
    wqkv_d = nc.dram_tensor("wqkv", [C, 576], BF16, kind="ExternalInput").ap()
    bqkv_d = nc.dram_tensor("bqkv", [576], F32, kind="ExternalInput").ap()
    wout_d = nc.dram_tensor("wout", [192, C], BF16, kind="ExternalInput").ap()
    y_d = nc.dram_tensor("y", [T, C], BF16, kind="ExternalOutput").ap()

    with tile.TileContext(nc) as tc:
        with tc.tile_pool(name="res", bufs=1) as res:
            QM = res.tile([128, HPC, T], BF16, tag="QM")
            KM = res.tile([128, HPC, T], BF16, tag="KM")
            v_st = res.tile([128, NKB, HPC, DH + 1], BF16, tag="v_st")
            wo01 = res.tile([128, C], BF16, tag="wo01")
            wo2 = res.tile([64, C], BF16, tag="wo2")
            at01 = res.tile([128, T], BF16, tag="at01")  # [h0 d | h1 d] x q
            at2 = res.tile([64, T], BF16, tag="at2")

            # 0/1 causal triangle for a diagonal 128-block (1.0 where q >= k)
            tri01 = res.tile([128, KB], BF16, tag="tri01")
            nc.gpsimd.memset(tri01[:], 1.0)
            nc.gpsimd.affine_select(
                out=tri01[:],
                in_=tri01[:],
                compare_op=mybir.AluOpType.is_ge,
                fill=0.0,
                base=0,
                pattern=[[1, KB]],
                channel_multiplier=-1,
            )
            # kill+triangle for the pair's upper member in the diagonal
            # super: cols 0-127 all-zero, cols 128-255 the triangle
            tri01x = res.tile([128, 2 * KB], BF16, tag="tri01x")
            nc.gpsimd.memset(tri01x[:], 1.0)
            nc.gpsimd.affine_select(
                out=tri01x[:],
                in_=tri01x[:],
                compare_op=mybir.AluOpType.is_ge,
                fill=0.0,
                base=-KB,
                pattern=[[1, 2 * KB]],
                channel_multiplier=-1,
            )
            nc.vector.memset(v_st[:, :, :, DH : DH + 1], 1.0)

            rep_ctx = (
                tc.For_i(0, cfg["repeat"], 1)
                if cfg.get("repeat", 1) > 1
                else contextlib.nullcontext()
            )
            rep_ctx.__enter__()

            # ---------------- Phase 1: projections ----------------
            with (
                tc.tile_pool(name="p1", bufs=1) as p1,
                tc.tile_pool(name="xts", bufs=3) as xpool,
                tc.tile_pool(name="pps", bufs=2, space="PSUM") as pps,
                tc.tile_pool(name="vps", bufs=2, space="PSUM") as vps,
            ):
                # 640 = 576 + 64 zero pad so the v-projection moving dim is 256
                wq_sb = p1.tile([128, 6, 640], BF16, tag="wq")
                bias_qk = p1.tile([128, 3], F32, tag="bqk")
                bias_v = p1.tile([128, 192], F32, tag="bv")
                bias_v_row = p1.tile([1, 192], F32, tag="bvr")

                for ci in range(6):
                    nc.sync.dma_start(
                        wq_sb[:, ci, 0:576],
                        wqkv_d[ci * 128 : (ci + 1) * 128, :],
                    )
                nc.vector.memset(wq_sb[:, :, 576:640], 0.0)
                # wout is needed only in phase 2 — load it behind the weights
                nc.sync.dma_start(wo01[:], wout_d[0:128, :])
                nc.sync.dma_start(wo2[:], wout_d[128:192, :])
                for m in range(3):
                    nc.sync.dma_start(
                        bias_qk[:, m : m + 1],
                        bqkv_d[m * 128 : (m + 1) * 128].rearrange("(p b) -> p b", b=1),
                    )
                nc.sync.dma_start(
                    bias_v_row[0:1, :],
                    bqkv_d[384:576].rearrange("(b f) -> b f", b=1),
                )
                nc.gpsimd.partition_broadcast(bias_v[:], bias_v_row[0:1, :])

                for ts in range(T // 512):
                    xts = xpool.tile([128, 6, 512], BF16, tag="xts")
                    for ci in range(6):
                        nc.sync.dma_start(
                            xts[:, ci, :],
                            xt_d[ci * 128 : (ci + 1) * 128,
                                 ts * 512 : (ts + 1) * 512],
                        )
                    col0 = ts * 512
                    # block h of the permuted weights is [q_h | k_h]: psum
                    # rows 0-63 = q_h^T, 64-127 = k_h^T
                    for h in range(3):
                        psq = pps.tile([128, 512], F32, tag="psq")
                        for ci in range(6):
                            nc.tensor.matmul(
                                psq[:],
                                wq_sb[:, ci, h * 128 : (h + 1) * 128],
                                xts[:, ci, :],
                                start=(ci == 0),
                                stop=(ci == 5),
                            )
                        nc.scalar.add(
                            QM[0:64, h, col0 : col0 + 512],
                            psq[0:64, :],
                            bias_qk[0:64, h : h + 1],
                        )
                        nc.scalar.add(
                            KM[64:128, h, col0 : col0 + 512],
                            psq[64:128, :],
                            bias_qk[64:128, h : h + 1],
                        )
                    # v in [token, d] layout: psum [tokens, 3*64 (+64 pad)]
                    for tb in range(4):
                        psv = vps.tile([128, 256], F32, tag="psv")
                        for ci in range(6):
                            nc.tensor.matmul(
                                psv[:],
                                xts[:, ci, tb * 128 : (tb + 1) * 128],
                                wq_sb[:, ci, 384:640],
                                start=(ci == 0),
                                stop=(ci == 5),
                            )
                        kb = ts * 4 + tb
                        nc.vector.tensor_tensor(
                            out=v_st[:, kb, :, 0:DH],
                            in0=psv[:, 0:192].rearrange("p (h d) -> p h d", h=HPC),
                            in1=bias_v[:].rearrange("p (h d) -> p h d", h=HPC),
                            op=mybir.AluOpType.add,
                        )

            # mirror each head's q/k onto the other partition half so paired
            # s^T matmuls stream identical SBUF addresses on both halves
            nc.sync.dma_start(QM[64:128, :, :], QM[0:64, :, :])
            nc.sync.dma_start(KM[0:64, :, :], KM[64:128, :, :])

            # ---------------- Phase 2: attention + out-projection ----------------
            with (
                tc.tile_pool(name="stps", bufs=cfg["st_bufs"], space="PSUM") as stps,
                tc.tile_pool(name="avps", bufs=1, space="PSUM") as avps,
                tc.tile_pool(name="ptp", bufs=cfg["pt_bufs"]) as ptp,
                tc.tile_pool(name="nrm", bufs=2) as nrm,
                tc.tile_pool(name="ysb", bufs=4) as ypool,
            ):

                def exp_chunk(st, pt, h, kb, qs, ext0, eng, hi_member):
                    """pt[:, ext0:SUP] = exp(SCALE * st[:, ext0:SUP]), then the
                    causal triangle kill for diagonal k-blocks (exp'd garbage
                    is finite positive, so 0/1 multiply masks it)."""
                    if cfg["skip_exp"]:
                        nc.vector.memset(pt[:, ext0 : ext0 + 2], 1.0)
                        return
                    if cfg.get("exp_eng"):
                        eng = cfg["exp_eng"]
                    if eng == "act":
                        nc.scalar.activation(
                            out=pt[:, ext0:SUP],
                            in_=st[:, ext0:SUP],
                            func=mybir.ActivationFunctionType.Exp,
                            bias=0.0,
                            scale=SCALE,
                        )
                    else:
                        nc.vector.tensor_scalar(
                            out=pt[:, ext0:SUP].bitcast(I16),
                            in0=st[:, ext0:SUP],
                            scalar1=float(SCH_A * SCALE),
                            scalar2=float(SCH_B),
                            op0=mybir.AluOpType.mult,
                            op1=mybir.AluOpType.add,
                        )
                    if kb >= qs * 8:  # diagonal k-block
                        mask = tri01x if hi_member else tri01
                        w = mask.shape[1]
                        nc.vector.tensor_tensor(
                            out=pt[:, ext0 : ext0 + w],
                            in0=pt[:, ext0 : ext0 + w],
                            in1=mask[:],
                            op=mybir.AluOpType.mult,
                        )

                def av_acc(av, pt, h, kb, qs, ext0):
                    nkb = (qs + 1) * (SUP // KB)
                    last_r0 = qs * 8 + 512 // KB - 1  # last kb touching cols [0,512)
                    c = ext0
                    while c < SUP:
                        ce = min((c // 512 + 1) * 512, SUP)
                        stop_kb = last_r0 if ce <= 512 else nkb - 1
                        nc.tensor.matmul(
                            av[:, c:ce],
                            v_st[:, kb, h, :],
                            pt[:, c:ce],
                            start=(kb == 0),
                            stop=(kb == stop_kb),
                        )
                        c = ce

                def norm_head(qs, h, av):
                    """rows 0-63 of av divided by row 64, into attnT storage."""
                    q0 = qs * SUP
                    stg = nrm.tile([65, SUP], F32, tag="stg")
                    nc.scalar.copy(stg[:], av[:])
                    den8 = nrm.tile([8, KB], F32, tag="den8")
                    nc.sync.dma_start(
                        den8[:],
                        stg[64:65, :].rearrange("r (a b) -> r a b", a=8),
                    )
                    rec8 = nrm.tile([8, KB], F32, tag="rec8")
                    nc.vector.reciprocal_approx_fast(out=rec8[:], in_=den8[:])
                    rec = nrm.tile([1, SUP], F32, tag="rec")
                    nc.sync.dma_start(
                        rec[0:1, :].rearrange("r (a b) -> r a b", a=8),
                        rec8[:],
                    )
                    recb = nrm.tile([64, SUP], F32, tag="recb")
                    nc.gpsimd.partition_broadcast(recb[:], rec[0:1, :])
                    if h == 0:
                        dest = at01[0:64, q0 : q0 + SUP]
                    elif h == 2:
                        dest = at2[:, q0 : q0 + SUP]
                    else:
                        # h1 rows belong at partitions 64-127 of at01; DVE
                        # can't shift partitions, so stage + DMA.
                        h1s = nrm.tile([64, SUP], BF16, tag="h1s")
                        dest = h1s[:]
                    nc.vector.tensor_tensor(
                        out=dest,
                        in0=stg[0:64, :],
                        in1=recb[:],
                        op=mybir.AluOpType.mult,
                    )
                    if h == 1:
                        nc.sync.dma_start(at01[64:128, q0 : q0 + SUP], h1s[:])

                def out_proj(qs):
                    q0 = qs * SUP
                    for tb in range(SUP // 128):
                        tcol = q0 + tb * 128
                        yps = stps.tile([128, SUP], F32, tag="st")
                        for rs, re in ((0, 512), (512, C)):
                            nc.tensor.matmul(
                                yps[:, rs:re],
                                at01[:, tcol : tcol + 128],
                                wo01[:, rs:re],
                                start=True,
                                stop=False,
                            )
                            nc.tensor.matmul(
                                yps[:, rs:re],
                                at2[:, tcol : tcol + 128],
                                wo2[:, rs:re],
                                start=False,
                                stop=True,
                            )
                        y_sb = ypool.tile([128, C], BF16, tag="ysb")
                        nc.scalar.copy(y_sb[:], yps[:, 0:C])
                        nc.sync.dma_start(y_d[tcol : tcol + 128, :], y_sb[:])

                def st_mm(st, h, kb, qs, ext0, half):
                    """s^T for one k-block on one partition half (row tile 0
                    or 64): [128 k, q] = KM-chunk^T @ QM. f32 PSUM output
                    limits one matmul to a 512-col bank."""
                    lo, hi = (0, 64) if half == 0 else (64, 128)
                    c = ext0
                    while c < SUP:
                        ce = min((c // 512 + 1) * 512, SUP)
                        nc.tensor.matmul(
                            st[:, c:ce],
                            KM[lo:hi, h, kb * KB : (kb + 1) * KB],
                            QM[lo:hi, h, qs * SUP + c : qs * SUP + ce],
                            start=True,
                            stop=True,
                        )
                        c = ce

                for qs in range(cfg["nsup"]) if not cfg["phase1_only"] else []:
                    nkb = (qs + 1) * (SUP // KB)
                    koff = qs * 8  # first diagonal k-block
                    for h in range(HPC):
                        av = avps.tile([65, SUP], F32, tag="av")
                        for r, kb in enumerate(range(0, nkb, 2)):
                            # pair (kb, kb+1): kb streams on partitions 0-63
                            # while kb+1 streams on 64-127, concurrently
                            t = kb - koff
                            ext0 = max(t, 0) * KB  # shared (lower member's)
                            stA = stps.tile([128, SUP], F32, tag="st")
                            stB = stps.tile([128, SUP], F32, tag="st")
                            st_mm(stA, h, kb, qs, ext0, 0)
                            st_mm(stB, h, kb + 1, qs, ext0, 1)
                            ptA = ptp.tile([128, SUP], BF16, tag="pt")
                            ptB = ptp.tile([128, SUP], BF16, tag="pt")
                            if qs == 0 and cfg["qs0_act"]:
                                e_lo = e_hi = "act"
                            elif r % 2 == 0:
                                e_lo, e_hi = cfg["eng_lo"], cfg["eng_hi"]
                            else:
                                e_lo, e_hi = cfg["eng_hi"], cfg["eng_lo"]
                            exp_chunk(stA, ptA, h, kb, qs, ext0, e_lo, False)
                            exp_chunk(stB, ptB, h, kb + 1, qs, ext0, e_hi, True)
                            av_acc(av, ptA, h, kb, qs, ext0)
                            av_acc(av, ptB, h, kb + 1, qs, ext0)
                        if not cfg["skip_norm"]:
                            norm_head(qs, h, av)
                    if qs >= 1 and not cfg["skip_outproj"]:
                        out_proj(qs - 1)
                if not cfg["phase1_only"] and not cfg["skip_outproj"]:
                    out_proj(cfg["nsup"] - 1)
            rep_ctx.__exit__(None, None, None)

    nc.compile()
    return nc


def shard_inputs(x, W_qkv, b_qkv, W_out, b_out):
    """Build the per-core input maps (host-side sharding)."""
    x = np.asarray(x, dtype=np.float32)
    W_qkv = np.asarray(W_qkv, dtype=np.float32)
    b_qkv = np.asarray(b_qkv, dtype=np.float32)
    W_out = np.asarray(W_out, dtype=np.float32)
    in_maps = []
    for c in range(NCORES):
        b = c // 4
        hh = (c % 4) * HPC

        def qcols(h):
            return list(range(h * DH, (h + 1) * DH))

        def kcols(h):
            return list(range(C + h * DH, C + (h + 1) * DH))

        def vcols(h):
            return list(range(2 * C + h * DH, 2 * C + (h + 1) * DH))

        perm = []
        for h in (hh, hh + 1, hh + 2):
            perm += qcols(h) + kcols(h)
        for h in (hh, hh + 1, hh + 2):
            perm += vcols(h)
        in_maps.append(
            {
                "xt": np.ascontiguousarray(x[b].T).astype(BF16_NP),
                "wqkv": np.ascontiguousarray(W_qkv[:, perm]).astype(BF16_NP),
                "bqkv": np.ascontiguousarray(b_qkv[perm]),
                "wout": np.ascontiguousarray(
                    W_out[hh * DH : (hh + HPC) * DH, :]
                ).astype(BF16_NP),
            }
        )
    return in_maps


def kernel(x, W_qkv, b_qkv, W_out, b_out):
    global _PROG, LAST_RESULT
    if _PROG is None:
        _PROG = build_program()
    nc = _PROG
    in_maps = shard_inputs(x, W_qkv, b_qkv, W_out, b_out)
    res = run_bass_kernel_spmd(nc, in_maps, list(range(NCORES)), trace=TRACE)
    LAST_RESULT = res
    b_out = np.asarray(b_out, dtype=np.float32)
    y = np.zeros((2, T, C), dtype=np.float32)
    for c in range(NCORES):
        y[c // 4] += res.results[c]["y"].astype(np.float32)
    y += b_out[None, None, :]
    return y
